# revision 18
# baseline (speedup 1.0000x reference)
"""Trainium2 Bass kernel for nn_CustomNetwork_37031208026716.

Network: 32 layers of (depth-1 butterfly rotation + interleave permutation +
smooth-bend activation y = u + cc*sqrt(u^2 + ik)) on X[65536, 512] fp32.

Strategy ("pair-compose", fp16 resident):
  * Pure data parallel over 8 cores (batch split, 8192 rows/core).
  * Width (512) on partitions as 4 tiles of 128; batch on the free axis,
    fp16 in SBUF.  Interleave permutation via conjugated coordinates:
    physical w at layer l is logical pi_l(w); butterfly pairs w with
    w^delta_l, delta_l = 2^((8-l)%9) (cross-tile when delta>=128).
  * Layer state is the PAIR (U, T): U = u (pre-bend affine value),
    T = sqrt(u^2+ik).  y = U + cc*T is only materialized (as Y, carrying a
    host-tracked additive offset) on layers feeding a cross-tile butterfly,
    so EVERY layer runs exactly two 128x128 fp16 matmul matrices:
      within-tile :  n = MU@U + MT@T      (MT folds prev layer's cc)
      cross-tile  :  n = D_s*Y_g + D_p*Y_g^  (diagonal matrices)
  * Remaining per-layer elementwise work, balanced across DVE and ACT:
      pair layers:  pull U' = n - pb  (DVE tensor_scalar from PSUM, or ACT
                    Identity from PSUM -- static split), q = U'*U' (DVE
                    tensor_tensor fp16 2x), T' = sqrt(q + ik) (ACT).
      ymat layers:  q = Square(n - pb) directly from PSUM (ACT),
                    T' = sqrt(q + ik) (ACT), Y' = cc*T' + n (DVE stt from
                    PSUM; carries offset pb which the host folds into the
                    next layer's biases).
  * Host casts fp32->fp16 for input/output transfers (halves HBM traffic).
"""

import numpy as np

BATCH = 65536
W = 512
HALF = 256
DEPTH = 32
NBITS = 9
NCORES = 8
NB = BATCH // NCORES          # batch rows per core
CH = 1024                     # batch columns per on-chip chunk
NTILE = 4                     # width tiles of 128 partitions
MMH = 512                     # moving free-dim per matmul (ISA cap)

CROSS = frozenset(l for l, d in enumerate(
    [1 << ((8 - l) % NBITS) for l in range(DEPTH)]) if d >= 128)
YMAT = frozenset(l for l in range(DEPTH) if (l + 1) in CROSS)

_P_ARR = np.array([(w >> 1) | ((w & 1) << 8) for w in range(W)], dtype=np.int64)


def _invert(p):
    inv = np.empty_like(p)
    inv[p] = np.arange(len(p))
    return inv


def _build_perms():
    pinv = _invert(_P_ARR)
    pis = [np.arange(W)]
    for l in range(DEPTH):
        pis.append(pinv[pis[l]])
    return pis


def _deltas():
    return [1 << ((8 - l) % NBITS) for l in range(DEPTH)]


def host_precompute(thetas, biases, slopes1, slopes2, curvatures):
    pis = _build_perms()
    thetas = thetas.astype(np.float64)
    c_all = np.cos(thetas)
    s_all = np.sin(thetas)
    m1 = np.exp(slopes1.astype(np.float64))
    m2 = np.exp(slopes2.astype(np.float64))
    a_all = (m1 + m2) * 0.5
    cc_all = (m2 - m1) / (2.0 * a_all)
    b_all = np.sinh(biases.astype(np.float64))
    ik_all = np.exp(-curvatures.astype(np.float64))

    idx_w = np.arange(W)
    layers = []
    ofs = np.zeros(W)                 # additive offset carried by Y tiles
    for l in range(DEPTH):
        pi_l, pi_n = pis[l], pis[l + 1]
        delta = 1 << ((8 - l) % NBITS)
        V = pi_l
        idx = V % HALF
        cA = c_all[l][idx]
        sA = np.where(V < HALF, s_all[l][idx], -s_all[l][idx])
        a_eff = a_all[l][pi_n]
        cAp = cA * a_eff
        sAp = sA * a_eff
        bb = b_all[l][pi_n]
        # effective pre-activation bias: subtract what the (offset) inputs
        # injected plus this layer's own bias
        pb = bb + cAp * ofs + sAp * ofs[idx_w ^ delta]
        ent = dict(delta=delta, cAp=cAp, sAp=sAp, pb=pb,
                   cc=cc_all[l][pi_n], ik=ik_all[l][pi_n])
        ofs = pb if l in YMAT else np.zeros(W)
        layers.append(ent)
    return layers, _invert(pis[DEPTH])


# ---- packing ---------------------------------------------------------------

def _wts_layout():
    out = []
    for l in range(DEPTH):
        for g in range(NTILE):
            if l in CROSS:
                out.append((l, g, "Ds"))
                out.append((l, g, "Dp"))
            else:
                out.append((l, g, "MU"))
                out.append((l, g, "MT"))
    return out


def pack_weights(layers):
    lay = _wts_layout()
    wts = np.zeros((128, len(lay) * 128), dtype=np.float16)
    p128 = np.arange(128)
    for j, (l, g, kind) in enumerate(lay):
        ent = layers[l]
        delta = ent["delta"]
        w = g * 128 + p128
        part = w ^ delta
        A = np.zeros((128, 128), dtype=np.float64)
        if kind == "Ds":
            A[p128, p128] = ent["cAp"][w]
        elif kind == "Dp":
            A[p128, p128] = ent["sAp"][w]
        elif kind == "MU":
            A[p128, p128] = ent["cAp"][w]
            A[p128, p128 ^ delta] = ent["sAp"][w]
        elif kind == "MT":
            prev = layers[l - 1]
            A[p128, p128] = ent["cAp"][w] * prev["cc"][w]
            A[p128, p128 ^ delta] = ent["sAp"][w] * prev["cc"][part]
        wts[:, j * 128:(j + 1) * 128] = A.T.astype(wts.dtype)
    return wts


_PKINDS = ("pb", "npb", "ik", "cc")
PRM_COLS = len(_PKINDS) * DEPTH * NTILE


def pack_params(layers):
    prm = np.zeros((128, PRM_COLS), dtype=np.float32)
    for l, ent in enumerate(layers):
        vals = dict(pb=ent["pb"], npb=-ent["pb"], ik=ent["ik"],
                    cc=ent["cc"])
        for k, kind in enumerate(_PKINDS):
            v = vals[kind]
            for g in range(NTILE):
                prm[:, (k * DEPTH + l) * NTILE + g] = v[g * 128:(g + 1) * 128]
    return prm


# ---- bass module -----------------------------------------------------------

def build_nc(nb=NB, ch=CH):
    from concourse import bacc, mybir
    from concourse.tile import TileContext

    f32 = mybir.dt.float32
    f16 = mybir.dt.float16
    AT = mybir.ActivationFunctionType
    OP = mybir.AluOpType

    deltas = _deltas()
    lay = _wts_layout()
    widx = {key: j for j, key in enumerate(lay)}

    nc = bacc.Bacc(target_bir_lowering=False, debug=False)
    xt = nc.declare_dram_parameter("xt", [W, nb], f16, isOutput=False)
    prm_d = nc.declare_dram_parameter("prm", [128, PRM_COLS], f32,
                                      isOutput=False)
    wts_d = nc.declare_dram_parameter("wts", [128, len(lay) * 128], f16,
                                      isOutput=False)
    out_d = nc.declare_dram_parameter("out", [W, nb], f16, isOutput=True)

    nchunk = nb // ch

    with TileContext(nc) as tc:
        with (
            tc.tile_pool(name="const", bufs=1) as cpool,
            tc.tile_pool(name="u", bufs=5) as upool,
            tc.tile_pool(name="t", bufs=5) as tpool,
            tc.tile_pool(name="q", bufs=6) as qpool,
            tc.tile_pool(name="y", bufs=5) as ypool,
            tc.tile_pool(name="ps", bufs=4, space="PSUM") as pspool,
        ):
            prm = cpool.tile([128, PRM_COLS], f32, tag="prm", name="prm")
            nc.sync.dma_start(out=prm, in_=prm_d[:, :])
            wsb = cpool.tile([128, len(lay) * 128], f16, tag="wts",
                             name="wsb")
            nw = len(lay) * 128
            for s in range(4):
                lo, hi = s * nw // 4, (s + 1) * nw // 4
                nc.sync.dma_start(out=wsb[:, lo:hi], in_=wts_d[:, lo:hi])

            def pcol(kind, l, g):
                k = _PKINDS.index(kind)
                c = (k * DEPTH + l) * NTILE + g
                return prm[:, c:c + 1]

            def lhs(l, g, kind):
                j = widx[(l, g, kind)]
                return wsb[:, j * 128:(j + 1) * 128]

            xt_r = xt.rearrange("(g p) b -> p g b", g=NTILE)
            out_r = out_d.rearrange("(g p) b -> p g b", g=NTILE)

            SUPER = 4
            for cp in range(nchunk // SUPER):
                cs = list(range(cp * SUPER, (cp + 1) * SUPER))
                # layer-0 inputs are Y tiles (offset 0)
                Ys, Us, Ts = {}, {}, {}
                for c in cs:
                    Ys[c] = {}
                    for g in range(NTILE):
                        Ys[c][g] = ypool.tile([128, ch], f16, tag=f"y{g}",
                                              name=f"y{g}")
                        nc.sync.dma_start(
                            out=Ys[c][g],
                            in_=xt_r[:, g, c * ch:(c + 1) * ch])
                    Us[c], Ts[c] = {}, {}

                for l in range(DEPTH):
                  delta = deltas[l]
                  qp, tp = {}, {}
                  for c in cs:
                    half = (c - cs[0]) % 2
                    Y, U, T = Ys[c], Us[c], Ts[c]
                    Un, Tn, Yn = {}, {}, {}
                    for g in range(NTILE):
                        ps = pspool.tile([128, ch], f32, tag="ps", name="ps")
                        for h in range(0, ch, MMH):
                            sl = slice(h, h + MMH)
                            if l in CROSS:
                                gp = g ^ (delta >> 7)
                                nc.tensor.matmul(
                                    ps[:, sl], lhs(l, g, "Ds"), Y[g][:, sl],
                                    start=True, stop=False)
                                nc.tensor.matmul(
                                    ps[:, sl], lhs(l, g, "Dp"), Y[gp][:, sl],
                                    start=False, stop=True)
                            else:
                                nc.tensor.matmul(
                                    ps[:, sl], lhs(l, g, "MU"), U[g][:, sl],
                                    start=True, stop=False)
                                nc.tensor.matmul(
                                    ps[:, sl], lhs(l, g, "MT"), T[g][:, sl],
                                    start=False, stop=True)
                        if l in YMAT:
                            # q = (n - pb)^2 straight from PSUM on ACT
                            q = qpool.tile([128, ch], f16, tag="qy",
                                           name="qy")
                            nc.scalar.activation(
                                q, ps, AT.Square, bias=pcol("npb", l, g),
                                scale=1.0)
                            t = tpool.tile([128, ch], f16, tag=f"ty{g}",
                                           name=f"ty{g}")
                            nc.scalar.activation(
                                t, q, AT.Sqrt, bias=pcol("ik", l, g),
                                scale=1.0)
                            y = ypool.tile([128, ch], f16, tag=f"y{g}",
                                           name=f"y{g}")
                            nc.vector.scalar_tensor_tensor(
                                y, t, pcol("cc", l, g), ps, OP.mult, OP.add)
                            Yn[g] = y
                            Ts[c][g] = t
                        else:
                            if half == 0:
                                qp[g] = qpool.tile([128, 2 * ch], f16,
                                                   tag=f"q{g}", name=f"q{g}")
                            qh = qp[g][:, half * ch:(half + 1) * ch]
                            u = upool.tile([128, ch], f16, tag=f"u{g}",
                                           name=f"u{g}")
                            if (l * NTILE + g + c) % 8 != 0:  # ~88% DVE pulls
                                nc.vector.tensor_scalar(
                                    u, ps, pcol("pb", l, g), None,
                                    OP.subtract)
                            else:
                                nc.scalar.activation(
                                    u, ps, AT.Identity,
                                    bias=pcol("npb", l, g), scale=1.0)
                            if (l + g + c) % 4 < 3:  # 75% squares on GpSimd
                                nc.gpsimd.tensor_tensor(qh, u, u, OP.mult)
                            else:
                                nc.vector.tensor_tensor(qh, u, u, OP.mult)
                            if half == 1:
                                tp[g] = tpool.tile([128, 2 * ch], f16,
                                                   tag=f"t{g}", name=f"t{g}")
                                nc.scalar.activation(
                                    tp[g], qp[g], AT.Sqrt,
                                    bias=pcol("ik", l, g), scale=1.0)
                            Un[g] = u
                            Tn[g] = True
                    if Un:
                        Us[c] = Un
                    if Yn:
                        Ys[c] = Yn
                  if l not in YMAT:
                    # wire paired-T slices to both chunks of each pair
                    for c in cs:
                        half = (c - cs[0]) % 2
                        for g in range(NTILE):
                            Ts[c][g] = tp[g][:, half * ch:(half + 1) * ch]

                for c in cs:
                    for g in range(NTILE):
                        y = ypool.tile([128, ch], f16, tag=f"y{g}",
                                       name=f"o{g}")
                        nc.vector.scalar_tensor_tensor(
                            y, Ts[c][g], pcol("cc", DEPTH - 1, g), Us[c][g],
                            OP.mult, OP.add)
                        nc.sync.dma_start(
                            out=out_r[:, g, c * ch:(c + 1) * ch], in_=y)
    nc.compile()
    return nc


_NC_CACHE = {}

TRACE = False
TRACE_KWARGS = {}
LAST_RESULTS = None


def _get_nc(nb, ch):
    key = (nb, ch)
    if key not in _NC_CACHE:
        _NC_CACHE[key] = build_nc(nb, ch)
    return _NC_CACHE[key]


def kernel(X, thetas, biases, slopes1, slopes2, curvatures):
    global LAST_RESULTS
    from concourse.bass_utils import run_bass_kernel_spmd

    X = np.asarray(X)
    layers, out_perm = host_precompute(
        np.asarray(thetas), np.asarray(biases), np.asarray(slopes1),
        np.asarray(slopes2), np.asarray(curvatures))
    prm = pack_params(layers)
    wts = pack_weights(layers)

    nc = _get_nc(NB, CH)
    in_maps = []
    for cid in range(NCORES):
        shard = np.ascontiguousarray(
            X[cid * NB:(cid + 1) * NB, :].T.astype(np.float16))
        in_maps.append({"xt": shard, "prm": prm, "wts": wts})

    res = run_bass_kernel_spmd(nc, in_maps, list(range(NCORES)),
                               trace=TRACE, **TRACE_KWARGS)
    LAST_RESULTS = res
    out = np.empty((BATCH, W), dtype=np.float32)
    for cid in range(NCORES):
        o = res.results[cid]["out"]          # [512, NB] fp16 physical order
        out[cid * NB:(cid + 1) * NB, :] = o[out_perm, :].T.astype(np.float32)
    return out


# revision 24
# speedup vs baseline: 1.0175x; 1.0175x over previous
"""Trainium2 Bass kernel for nn_CustomNetwork_37031208026716.

Network: 32 layers of (depth-1 butterfly rotation + interleave permutation +
smooth-bend activation y = u + cc*sqrt(u^2 + ik)) on X[65536, 512] fp32.

Strategy ("pair-compose", fp16 resident):
  * Pure data parallel over 8 cores (batch split, 8192 rows/core).
  * Width (512) on partitions as 4 tiles of 128; batch on the free axis,
    fp16 in SBUF.  Interleave permutation via conjugated coordinates:
    physical w at layer l is logical pi_l(w); butterfly pairs w with
    w^delta_l, delta_l = 2^((8-l)%9) (cross-tile when delta>=128).
  * Layer state is the PAIR (U, T): U = u (pre-bend affine value),
    T = sqrt(u^2+ik).  y = U + cc*T is only materialized (as Y, carrying a
    host-tracked additive offset) on layers feeding a cross-tile butterfly,
    so EVERY layer runs exactly two 128x128 fp16 matmul matrices:
      within-tile :  n = MU@U + MT@T      (MT folds prev layer's cc)
      cross-tile  :  n = D_s*Y_g + D_p*Y_g^  (diagonal matrices)
  * Remaining per-layer elementwise work, balanced across DVE/ACT/GpSimd:
      pair layers:  pull U' = n - pb  (DVE tensor_scalar from PSUM; ~6% on
                    ACT Identity), q = U'*U' (75% GpSimd tensor_tensor, 25%
                    DVE fp16 2x), T' = sqrt(q + ik) (ACT, per-partition ik).
      ymat layers:  q = Square(n - pb) directly from PSUM (ACT),
                    T' = sqrt(q + ik) (ACT), Y' = cc*T' + n (DVE stt from
                    PSUM; carries offset pb which the host folds into the
                    next layer's biases).
  * Stationary matrices in bf16 (PE cost is set by the fp16 moving operand;
    bf16 weights keep rel-err ~1e-2, well under the 2e-2 gate).
  * Four 1024-column chunks interleaved per pass to keep ~16 tile-streams
    in flight (hides the cross-engine mm->pull->square->sqrt latency).
  * Host casts fp32->fp16 for input/output transfers (halves HBM traffic).
"""

import numpy as np

BATCH = 65536
W = 512
HALF = 256
DEPTH = 32
NBITS = 9
NCORES = 8
NB = BATCH // NCORES          # batch rows per core
CH = 1024                     # batch columns per on-chip chunk
NTILE = 4                     # width tiles of 128 partitions
MMH = 512                     # moving free-dim per matmul (ISA cap)

CROSS = frozenset(l for l, d in enumerate(
    [1 << ((8 - l) % NBITS) for l in range(DEPTH)]) if d >= 128)
YMAT = frozenset(l for l in range(DEPTH) if (l + 1) in CROSS)

_P_ARR = np.array([(w >> 1) | ((w & 1) << 8) for w in range(W)], dtype=np.int64)


def _invert(p):
    inv = np.empty_like(p)
    inv[p] = np.arange(len(p))
    return inv


def _build_perms():
    pinv = _invert(_P_ARR)
    pis = [np.arange(W)]
    for l in range(DEPTH):
        pis.append(pinv[pis[l]])
    return pis


def _deltas():
    return [1 << ((8 - l) % NBITS) for l in range(DEPTH)]


def host_precompute(thetas, biases, slopes1, slopes2, curvatures):
    pis = _build_perms()
    thetas = thetas.astype(np.float64)
    c_all = np.cos(thetas)
    s_all = np.sin(thetas)
    m1 = np.exp(slopes1.astype(np.float64))
    m2 = np.exp(slopes2.astype(np.float64))
    a_all = (m1 + m2) * 0.5
    cc_all = (m2 - m1) / (2.0 * a_all)
    b_all = np.sinh(biases.astype(np.float64))
    ik_all = np.exp(-curvatures.astype(np.float64))

    idx_w = np.arange(W)
    layers = []
    ofs = np.zeros(W)                 # additive offset carried by Y tiles
    for l in range(DEPTH):
        pi_l, pi_n = pis[l], pis[l + 1]
        delta = 1 << ((8 - l) % NBITS)
        V = pi_l
        idx = V % HALF
        cA = c_all[l][idx]
        sA = np.where(V < HALF, s_all[l][idx], -s_all[l][idx])
        a_eff = a_all[l][pi_n]
        cAp = cA * a_eff
        sAp = sA * a_eff
        bb = b_all[l][pi_n]
        # effective pre-activation bias: subtract what the (offset) inputs
        # injected plus this layer's own bias
        pb = bb + cAp * ofs + sAp * ofs[idx_w ^ delta]
        ent = dict(delta=delta, cAp=cAp, sAp=sAp, pb=pb,
                   cc=cc_all[l][pi_n], ik=ik_all[l][pi_n])
        ofs = pb if l in YMAT else np.zeros(W)
        layers.append(ent)
    return layers, _invert(pis[DEPTH])


# ---- packing ---------------------------------------------------------------

def _wts_layout():
    out = []
    for l in range(DEPTH):
        for g in range(NTILE):
            if l in CROSS:
                out.append((l, g, "Ds"))
                out.append((l, g, "Dp"))
            else:
                out.append((l, g, "MU"))
                out.append((l, g, "MT"))
    return out


def pack_weights(layers):
    import ml_dtypes
    lay = _wts_layout()
    wts = np.zeros((128, len(lay) * 128), dtype=ml_dtypes.bfloat16)
    p128 = np.arange(128)
    for j, (l, g, kind) in enumerate(lay):
        ent = layers[l]
        delta = ent["delta"]
        w = g * 128 + p128
        part = w ^ delta
        A = np.zeros((128, 128), dtype=np.float64)
        if kind == "Ds":
            A[p128, p128] = ent["cAp"][w]
        elif kind == "Dp":
            A[p128, p128] = ent["sAp"][w]
        elif kind == "MU":
            A[p128, p128] = ent["cAp"][w]
            A[p128, p128 ^ delta] = ent["sAp"][w]
        elif kind == "MT":
            prev = layers[l - 1]
            A[p128, p128] = ent["cAp"][w] * prev["cc"][w]
            A[p128, p128 ^ delta] = ent["sAp"][w] * prev["cc"][part]
        wts[:, j * 128:(j + 1) * 128] = A.T.astype(wts.dtype)
    return wts


_PKINDS = ("pb", "npb", "ik", "cc")
PRM_COLS = len(_PKINDS) * DEPTH * NTILE


def pack_params(layers):
    prm = np.zeros((128, PRM_COLS), dtype=np.float32)
    for l, ent in enumerate(layers):
        vals = dict(pb=ent["pb"], npb=-ent["pb"], ik=ent["ik"],
                    cc=ent["cc"])
        for k, kind in enumerate(_PKINDS):
            v = vals[kind]
            for g in range(NTILE):
                prm[:, (k * DEPTH + l) * NTILE + g] = v[g * 128:(g + 1) * 128]
    return prm


# ---- bass module -----------------------------------------------------------

def build_nc(nb=NB, ch=CH):
    from concourse import bacc, mybir
    from concourse.tile import TileContext

    f32 = mybir.dt.float32
    f16 = mybir.dt.float16
    AT = mybir.ActivationFunctionType
    OP = mybir.AluOpType

    deltas = _deltas()
    lay = _wts_layout()
    widx = {key: j for j, key in enumerate(lay)}

    nc = bacc.Bacc(target_bir_lowering=False, debug=False)
    xt = nc.declare_dram_parameter("xt", [W, nb], f16, isOutput=False)
    prm_d = nc.declare_dram_parameter("prm", [128, PRM_COLS], f32,
                                      isOutput=False)
    wts_d = nc.declare_dram_parameter("wts", [128, len(lay) * 128],
                                      mybir.dt.bfloat16, isOutput=False)
    out_d = nc.declare_dram_parameter("out", [W, nb], f16, isOutput=True)

    nchunk = nb // ch

    with TileContext(nc) as tc:
        with (
            tc.tile_pool(name="const", bufs=1) as cpool,
            tc.tile_pool(name="u", bufs=5) as upool,
            tc.tile_pool(name="t", bufs=5) as tpool,
            tc.tile_pool(name="q", bufs=6) as qpool,
            tc.tile_pool(name="y", bufs=5) as ypool,
            tc.tile_pool(name="ps", bufs=4, space="PSUM") as pspool,
        ):
            prm = cpool.tile([128, PRM_COLS], f32, tag="prm", name="prm")
            nc.sync.dma_start(out=prm, in_=prm_d[:, :])
            wsb = cpool.tile([128, len(lay) * 128], mybir.dt.bfloat16,
                             tag="wts", name="wsb")
            nw = len(lay) * 128
            for s in range(4):
                lo, hi = s * nw // 4, (s + 1) * nw // 4
                nc.sync.dma_start(out=wsb[:, lo:hi], in_=wts_d[:, lo:hi])

            def pcol(kind, l, g):
                k = _PKINDS.index(kind)
                c = (k * DEPTH + l) * NTILE + g
                return prm[:, c:c + 1]

            def lhs(l, g, kind):
                j = widx[(l, g, kind)]
                return wsb[:, j * 128:(j + 1) * 128]

            xt_r = xt.rearrange("(g p) b -> p g b", g=NTILE)
            out_r = out_d.rearrange("(g p) b -> p g b", g=NTILE)

            SUPER = 4
            for cp in range(nchunk // SUPER):
                cs = list(range(cp * SUPER, (cp + 1) * SUPER))
                # layer-0 inputs are Y tiles (offset 0)
                Ys, Us, Ts = {}, {}, {}
                for c in cs:
                    Ys[c] = {}
                    for g in range(NTILE):
                        Ys[c][g] = ypool.tile([128, ch], f16, tag=f"y{g}",
                                              name=f"y{g}")
                        nc.sync.dma_start(
                            out=Ys[c][g],
                            in_=xt_r[:, g, c * ch:(c + 1) * ch])
                    Us[c], Ts[c] = {}, {}

                for l in range(DEPTH):
                  delta = deltas[l]
                  for c in cs:
                    Y, U, T = Ys[c], Us[c], Ts[c]
                    Un, Tn, Yn = {}, {}, {}
                    for g in range(NTILE):
                        ps = pspool.tile([128, ch], f32, tag="ps", name="ps")
                        for h in range(0, ch, MMH):
                            sl = slice(h, h + MMH)
                            if l in CROSS:
                                gp = g ^ (delta >> 7)
                                nc.tensor.matmul(
                                    ps[:, sl], lhs(l, g, "Ds"), Y[g][:, sl],
                                    start=True, stop=False)
                                nc.tensor.matmul(
                                    ps[:, sl], lhs(l, g, "Dp"), Y[gp][:, sl],
                                    start=False, stop=True)
                            else:
                                nc.tensor.matmul(
                                    ps[:, sl], lhs(l, g, "MU"), U[g][:, sl],
                                    start=True, stop=False)
                                nc.tensor.matmul(
                                    ps[:, sl], lhs(l, g, "MT"), T[g][:, sl],
                                    start=False, stop=True)
                        if l in YMAT:
                            # q = (n - pb)^2 straight from PSUM on ACT
                            q = qpool.tile([128, ch], f16, tag="q", name="q")
                            nc.scalar.activation(
                                q, ps, AT.Square, bias=pcol("npb", l, g),
                                scale=1.0)
                            t = tpool.tile([128, ch], f16, tag=f"t{g}",
                                           name=f"t{g}")
                            nc.scalar.activation(
                                t, q, AT.Sqrt, bias=pcol("ik", l, g),
                                scale=1.0)
                            y = ypool.tile([128, ch], f16, tag=f"y{g}",
                                           name=f"y{g}")
                            nc.vector.scalar_tensor_tensor(
                                y, t, pcol("cc", l, g), ps, OP.mult, OP.add)
                            Yn[g] = y
                        else:
                            u = upool.tile([128, ch], f16, tag=f"u{g}",
                                           name=f"u{g}")
                            if (l * NTILE + g + c) % 16 != 0:
                                nc.vector.tensor_scalar(
                                    u, ps, pcol("pb", l, g), None,
                                    OP.subtract)
                            else:
                                nc.scalar.activation(
                                    u, ps, AT.Identity,
                                    bias=pcol("npb", l, g), scale=1.0)
                            q = qpool.tile([128, ch], f16, tag="q", name="q")
                            if (l + g + c) % 4 < 3:  # 75% squares on GpSimd
                                nc.gpsimd.tensor_tensor(q, u, u, OP.mult)
                            else:
                                nc.vector.tensor_tensor(q, u, u, OP.mult)
                            t = tpool.tile([128, ch], f16, tag=f"t{g}",
                                           name=f"t{g}")
                            nc.scalar.activation(
                                t, q, AT.Sqrt, bias=pcol("ik", l, g),
                                scale=1.0)
                            Un[g] = u
                            Tn[g] = t
                    if Yn:
                        Ys[c] = Yn
                    Us[c] = Un or U
                    Ts[c] = Tn or T

                for c in cs:
                    for g in range(NTILE):
                        y = ypool.tile([128, ch], f16, tag=f"y{g}",
                                       name=f"o{g}")
                        nc.vector.scalar_tensor_tensor(
                            y, Ts[c][g], pcol("cc", DEPTH - 1, g), Us[c][g],
                            OP.mult, OP.add)
                        nc.sync.dma_start(
                            out=out_r[:, g, c * ch:(c + 1) * ch], in_=y)
    nc.compile()
    return nc


_NC_CACHE = {}

TRACE = False
TRACE_KWARGS = {}
LAST_RESULTS = None


def _get_nc(nb, ch):
    key = (nb, ch)
    if key not in _NC_CACHE:
        _NC_CACHE[key] = build_nc(nb, ch)
    return _NC_CACHE[key]


def kernel(X, thetas, biases, slopes1, slopes2, curvatures):
    global LAST_RESULTS
    from concourse.bass_utils import run_bass_kernel_spmd

    X = np.asarray(X)
    layers, out_perm = host_precompute(
        np.asarray(thetas), np.asarray(biases), np.asarray(slopes1),
        np.asarray(slopes2), np.asarray(curvatures))
    prm = pack_params(layers)
    wts = pack_weights(layers)

    nc = _get_nc(NB, CH)
    in_maps = []
    for cid in range(NCORES):
        shard = np.ascontiguousarray(
            X[cid * NB:(cid + 1) * NB, :].T.astype(np.float16))
        in_maps.append({"xt": shard, "prm": prm, "wts": wts})

    res = run_bass_kernel_spmd(nc, in_maps, list(range(NCORES)),
                               trace=TRACE, **TRACE_KWARGS)
    LAST_RESULTS = res
    out = np.empty((BATCH, W), dtype=np.float32)
    for cid in range(NCORES):
        o = res.results[cid]["out"]          # [512, NB] fp16 physical order
        out[cid * NB:(cid + 1) * NB, :] = o[out_perm, :].T.astype(np.float32)
    return out
